# revision 34
# baseline (speedup 1.0000x reference)
"""Multi-head distance (attention) layer on 8 TRN2 NeuronCores.

Sharding: data-parallel over batch. B=8 -> one batch element per core.
Each core computes a full multi-head self-attention for its [L=1024, D=256]
slice with H=8 heads of dim 64. No collectives needed.

The kernel is ScalarE(ACT)-throughput-bound: softmax needs exp on all
H*L*L = 8.4M scores and ACT is the only engine with exp, at 128 lanes x
1.2 GHz => ~55us floor.  Everything is organized around keeping ACT 100%
busy doing nothing but exp:
  - The host ships x already TRANSPOSED, both with the positional
    encoding added in fp32 (qkpT = (x+pe).T, feeds Q/K) and without
    (xT, feeds V).  That kills the on-device transposes and pos-enc
    adds and makes every input DMA a single large contiguous transfer
    (one InstDMACopy stripes across all 16 SDMA engines).
  - PSUM: 6 banks are an S-score rotation (2 tiles x [128, 1536] fp32,
    i.e. 3 matmul chunks of 512 per exp) so each ACTIVATE amortizes its
    352-cycle fixed overhead over 1536 elements; the other 2 banks are a
    shared pool for QKV-projection / O-quad matmul outputs.
  - ACT executes ONLY the exp stream (plus input DMA triggers and one
    table preload, all finished before the first exp); every PSUM
    drain/copy lives on DVE (gpsimd/Pool cannot touch PSUM).
  - S matmuls use 64x128 PE row-tiling (tile_position): head 2j streams
    through PE rows 0-63 while head 2j+1 streams rows 64-127
    concurrently, so the d=64 contraction runs at full rate with no
    zero-padded K and no zeroed kT copies.
  - The S/exp stream is software-pipelined with the O quads of the
    previous head pair at half-S-pair granularity so the in-order PE
    queue never runs ACT dry; only the last two O quads (which consume
    the final exps) trail the exp stream.
Per-core algorithm (all matmul operands fp16: 1 col/cycle on the PE with
~fp32-grade mantissa for this problem's value ranges):
  qT   = Wq.T @ qkpT + bq       via matmul(lhsT=Wq, rhs=qkpT), DVE drain
  kTp  = Wk.T @ qkpT            per head-pair [128=2x64 d, 1024 m]
  v    = xT.T @ Wv              via matmul(lhsT=xT, rhs=Wv)
  per head pair (2j, 2j+1), interleaved chunk stream:
    sT[m,l] = sum_d kTp[d,m] qT[d,l]     row-tiled matmul chunks
    eT      = exp(0.125 * sT)            ACT, PSUM->SBUF, fp16, [128,1536]
    O[l,d]+Z = eT.T @ [v_h | 1]          matmul(lhsT=eT, rhs=v_aug), 4 output
                                         column-groups share one PSUM bank
    out_h   = O * (1/Z)                  DVE reciprocal + broadcast multiply,
                                         fp16 out_sb, DMA'd out per quad
Bias handling: bq added to qT during PSUM drain (per-partition scalar);
bk only shifts each score row by a constant (softmax-invariant) so it is
dropped; bv shifts the output by exactly repeat(bv, 64) because softmax
rows sum to 1, added on the host (which also upcasts the fp16 result).
"""

import numpy as np

import concourse.bass as bass
import concourse.mybir as mybir
import concourse.tile as tile
from concourse import bacc
from concourse.bass_utils import run_bass_kernel_spmd

B, L, D = 8, 1024, 256
H, HD = 8, 64
J = H * HD  # 512
TEMPERATURE = 10000.0

f32 = mybir.dt.float32
bf16 = mybir.dt.float16  # fp16: same PE rate as bf16, 8x the mantissa

_CACHE = {}
LAST_RESULT = None  # BassKernelResults of the most recent run (for profiling)
TRACE = False

STILE = 1536  # S-chunk PSUM/exp tile width (3 chunks of 512)


def _emit(tc, aps):
    nc = tc.nc
    Exp = mybir.ActivationFunctionType.Exp
    qkp, xt, wq, wk, wv, bqc, out = (
        aps["qkp"], aps["xt"], aps["wq"], aps["wk"], aps["wv"], aps["bqc"],
        aps["out"],
    )

    # all inputs arrive partition-major from the host: one contiguous
    # 2-4KB run per partition -> one InstDMACopy each with max-size
    # descriptors striped over the 16 SDMA engines.  qkp additionally
    # arrives l-half-major ([p][(lhalf, t, 512)]) so the first projection
    # pieces unblock after half the transfer.
    qkpr = qkp.rearrange("p (h t l) -> p h t l", t=2, l=512)  # [128,2,2,512]
    xtr = xt.rearrange("p (t l) -> p t l", l=1024)       # [128, 2, 1024]
    wqr = wq.rearrange("p (t j) -> p t j", j=512)        # [128, 2, 512]
    wkr = wk.rearrange("p (t j) -> p t j", j=512)
    wvr = wv.rearrange("p (t j) -> p t j", j=512)
    outr = out.rearrange("(n p) j -> p n j", p=128)      # [128, 8, 512]

    import contextlib
    ctx = contextlib.ExitStack()
    persist = ctx.enter_context(tc.tile_pool(name="persist", bufs=1))
    epool = ctx.enter_context(tc.tile_pool(name="epool", bufs=16))
    rpool = ctx.enter_context(tc.tile_pool(name="rpool", bufs=4))
    s_ps = ctx.enter_context(tc.tile_pool(name="sps", bufs=2, space="PSUM"))
    o_ps = ctx.enter_context(tc.tile_pool(name="ops", bufs=2, space="PSUM"))

    # --- input DMAs: one large contiguous transfer each, spread over the
    # three DMA-capable queues, critical-path operands (qkp, wk, wq) first.
    # qkp is split by L-half so the first k/q projection pieces (which only
    # read l 0:512) fire one transfer earlier.
    qkT = persist.tile([128, 2, 1024], bf16, name="qkT")
    xT = persist.tile([128, 2, 1024], bf16, name="xT")
    w_sb = {
        wname: persist.tile([128, 2, 512], bf16, name=f"{wname}_sb")
        for wname in ("wq", "wk", "wv")
    }
    bq_sb = persist.tile([128, 4], f32, name="bq_sb")

    # qkT SBUF layout is [128, t, l]; the lo DMA fills [:, :, 0:512] from
    # the contiguous lo block, hi fills [:, :, 512:1024]
    nc.sync.dma_start(out=qkT[:, :, 0:512], in_=qkpr[:, 0])
    nc.scalar.dma_start(out=w_sb["wk"][:], in_=wkr[:])
    nc.gpsimd.dma_start(out=w_sb["wq"][:], in_=wqr[:])
    nc.sync.dma_start(out=qkT[:, :, 512:1024], in_=qkpr[:, 1])
    nc.gpsimd.dma_start(out=bq_sb[:], in_=bqc[:, :])
    nc.scalar.dma_start(out=w_sb["wv"][:], in_=wvr[:])
    nc.sync.dma_start(out=xT[:], in_=xtr[:])

    # --- ACT exp-table preload (after ACT's DMA triggers, before first exp)
    sc_in = persist.tile([128, 8], f32, name="sc_in")
    sc_out = persist.tile([128, 8], f32, name="sc_out")
    nc.vector.memset(sc_in[:], 0.0)
    nc.scalar.activation(sc_out[:], sc_in[:], Exp)

    # --- QKV projections (o-pool PSUM, drains on DVE) ---
    kTp = [persist.tile([128, 1024], bf16, name=f"kTp{j}") for j in range(4)]
    qT = [persist.tile([128, 1024], bf16, name=f"qT{j}") for j in range(4)]
    v_sb = [persist.tile([128, 8, 65], bf16, name=f"v_sb{m}") for m in range(8)]

    def qk_piece(j, which, l2):
        wname = "wq" if which == "q" else "wk"
        pq = o_ps.tile([128, 512], f32, tag="o", name="pq")
        for c2 in range(2):
            nc.tensor.matmul(
                pq[:, 0:512],
                lhsT=w_sb[wname][:, c2, j * 128:(j + 1) * 128],
                rhs=qkT[:, c2, l2 * 512:(l2 + 1) * 512],
                start=(c2 == 0),
                stop=(c2 == 1),
            )
        dsl = slice(l2 * 512, (l2 + 1) * 512)
        if which == "q":
            nc.vector.tensor_scalar_add(
                qT[j][:, dsl], pq[:, 0:512], bq_sb[:, j:j + 1]
            )
        else:
            nc.vector.tensor_copy(kTp[j][:, dsl], pq[:, 0:512])

    def v_proj(m):
        pv = o_ps.tile([128, 512], f32, tag="o", name="pv")
        for c2 in range(2):
            nc.tensor.matmul(
                pv[:, 0:512],
                lhsT=xT[:, c2, m * 128:(m + 1) * 128],
                rhs=w_sb["wv"][:, c2, :],
                start=(c2 == 0),
                stop=(c2 == 1),
            )
        nc.vector.tensor_copy(
            v_sb[m][:, :, 0:64], pv[:, 0:512].rearrange("p (h d) -> p h d", h=8)
        )
        nc.vector.memset(v_sb[m][:, :, 64:65], 1.0)

    # --- attention: S-chunks packed into [128, STILE] PSUM tiles; one exp
    # per tile. ---
    out_sb = persist.tile([128, 8, 512], bf16, name="out_sb")
    epos = {}  # (h, mc, l2) -> (e_tile, col_offset)
    state = {"tile": None, "off": 0, "chunks": []}

    def flush_exp():
        if state["tile"] is None or not state["chunks"]:
            return
        e = epool.tile([128, state["off"]], bf16, tag="e", name="e")
        nc.scalar.activation(
            e[:], state["tile"][:, 0:state["off"]], Exp, scale=float(HD) ** -0.5
        )
        for key, off in state["chunks"]:
            epos[key] = (e, off)
        state["tile"] = None
        state["off"] = 0
        state["chunks"] = []

    def s_chunk(h, mc, l2):
        if state["tile"] is None:
            state["tile"] = s_ps.tile([128, STILE], f32, tag="s", name="ps")
        off = state["off"]
        j, half = h >> 1, h & 1
        hp = slice(half * 64, half * 64 + 64)
        nc.tensor.matmul(
            state["tile"][:, off:off + 512],
            lhsT=kTp[j][hp, mc * 128:(mc + 1) * 128],
            rhs=qT[j][hp, l2 * 512:(l2 + 1) * 512],
            start=True,
            stop=True,
            tile_position=(half * 64, 0),
        )
        state["chunks"].append(((h, mc, l2), off))
        state["off"] = off + 512
        if state["off"] == STILE:
            flush_exp()

    def emit_S_sub(p, l2, mcs, heads=None):
        # heads=None: pair-interleaved (row-tiled concurrency); otherwise a
        # single head's chunks (used for the final pair so O(6,1) is gated
        # on an exp three tiles before the last one)
        for mc in mcs:
            for h in ([2 * p, 2 * p + 1] if heads is None else heads):
                s_chunk(h, mc, l2)

    pO_open = {}

    def emit_O_mms(h, q, gs, pO):
        for g in gs:
            lc = 4 * q + g
            l2, sub = lc // 4, lc % 4
            for mc in range(8):
                e, off = epos[(h, mc, l2)]
                nc.tensor.matmul(
                    pO[:, 65 * g:65 * g + 65],
                    lhsT=e[:, off + sub * 128:off + (sub + 1) * 128],
                    rhs=v_sb[mc][:, h, :],
                    start=(mc == 0),
                    stop=(mc == 7),
                )

    def emit_O_half(h, q, part):
        if part == 0:
            pO = o_ps.tile([128, 260], f32, tag="o", name="pO")
            pO_open[(h, q)] = pO
            emit_O_mms(h, q, (0, 1), pO)
        else:
            pO = pO_open.pop((h, q))
            emit_O_mms(h, q, (2, 3), pO)
            emit_O_fin(h, q, pO)

    def emit_O_fin(h, q, pO):
        hsl = slice(h * 64, (h + 1) * 64)
        pOr = pO.rearrange("p (g c) -> p g c", g=4)      # [128, 4, 65]
        rc = rpool.tile([128, 4], f32, tag="rc", name="rc")
        nc.vector.reciprocal(rc[:], pOr[:, :, 64])
        rcb = bass.AP(
            tensor=rc.tensor, offset=rc.offset,
            ap=[rc.ap[0], rc.ap[1], [0, 64]],
        )
        nc.vector.tensor_mul(
            out_sb[:, 4 * q:4 * q + 4, hsl], pOr[:, :, 0:64], rcb
        )
        if h == 7 and q == 1:
            for g2, eng in ((slice(4, 6), nc.sync), (slice(6, 8), nc.scalar)):
                eng.dma_start(out=outr[:, g2, hsl], in_=out_sb[:, g2, hsl])
        elif h == 7:
            for g2, eng in ((slice(0, 2), nc.sync), (slice(2, 4), nc.gpsimd)):
                eng.dma_start(out=outr[:, g2, hsl], in_=out_sb[:, g2, hsl])
        elif h == 6 and q == 1:
            for g2, eng in ((slice(4, 6), nc.sync), (slice(6, 8), nc.scalar)):
                eng.dma_start(out=outr[:, g2, hsl], in_=out_sb[:, g2, hsl])
        else:
            eng = nc.sync if q == 0 else nc.gpsimd
            eng.dma_start(
                out=outr[:, 4 * q:4 * q + 4, hsl],
                in_=out_sb[:, 4 * q:4 * q + 4, hsl],
            )

    def emit_O_quad(h, q):
        pO = o_ps.tile([128, 260], f32, tag="o", name="pO")
        emit_O_mms(h, q, (0, 1, 2, 3), pO)
        emit_O_fin(h, q, pO)

    # schedule: S emission in 4-chunk sub-steps with the previous pair's O
    # half-quads and the next pair's QKV projections dropped in between, so
    # the in-order PE queue always has S fills near the head and ACT never
    # runs dry.  The final pair's l2=1 chunks go head-6-first so O(6,1)
    # unblocks three exps before the end; only O(7,1) trails the final exp.
    qk_piece(0, "k", 0)
    qk_piece(0, "q", 0)
    emit_S_sub(0, 0, (0, 1))
    qk_piece(0, "k", 1)
    emit_S_sub(0, 0, (2, 3))
    qk_piece(0, "q", 1)
    emit_S_sub(0, 0, (4, 5))
    qk_piece(1, "k", 0)
    emit_S_sub(0, 0, (6, 7))
    qk_piece(1, "k", 1)
    emit_S_sub(0, 1, (0, 1))
    v_proj(0)
    emit_S_sub(0, 1, (2, 3))
    v_proj(1)
    qk_piece(1, "q", 0)
    emit_S_sub(0, 1, (4, 5))
    v_proj(2)
    qk_piece(1, "q", 1)
    emit_S_sub(0, 1, (6, 7))
    for m in range(3, 8):
        v_proj(m)
    for p in range(1, 4):
        quads = [(2 * p - 2, 0), (2 * p - 2, 1), (2 * p - 1, 0), (2 * p - 1, 1)]
        inj = [("k", 0), ("k", 1), ("q", 0), ("q", 1)] if p < 3 else None
        subs = (
            [(0, (0, 1), None), (0, (2, 3), None), (0, (4, 5), None),
             (0, (6, 7), None), (1, (0, 1), None), (1, (2, 3), None),
             (1, (4, 5), None), (1, (6, 7), None)]
            if p < 3 else
            [(0, (0, 1), None), (0, (2, 3), None), (0, (4, 5), None),
             (0, (6, 7), None), (1, (0, 1, 2, 3), [6]), (1, (4, 5, 6, 7), [6]),
             (1, (0, 1, 2, 3), [7]), (1, (4, 5, 6, 7), [7])]
        )
        for subi, (l2, mcs, heads) in enumerate(subs):
            emit_S_sub(p, l2, mcs, heads)
            h, q = quads[subi // 2]
            emit_O_half(h, q, subi % 2)
            if inj is not None and subi % 2 == 1:
                w, l2i = inj[subi // 2]
                qk_piece(p + 1, w, l2i)
    # O(6,0)/O(7,0) consume l2=0 exps (long done) and run during the final
    # exps; O(6,1) is gated three exps early, O(7,1) on the final exp.
    flush_exp()
    emit_O_quad(6, 0)
    emit_O_quad(7, 0)
    emit_O_quad(6, 1)
    emit_O_quad(7, 1)
    ctx.close()


def _build():
    if "nc" in _CACHE:
        return _CACHE["nc"]
    nc = bacc.Bacc("TRN2", target_bir_lowering=False, debug=False, num_devices=8)
    aps = {
        "qkp": nc.dram_tensor("qkp", [128, 2 * L], bf16, kind="ExternalInput").ap(),
        "xt": nc.dram_tensor("xt", [128, 2 * L], bf16, kind="ExternalInput").ap(),
        "wq": nc.dram_tensor("wq", [128, 2 * J], bf16, kind="ExternalInput").ap(),
        "wk": nc.dram_tensor("wk", [128, 2 * J], bf16, kind="ExternalInput").ap(),
        "wv": nc.dram_tensor("wv", [128, 2 * J], bf16, kind="ExternalInput").ap(),
        "bqc": nc.dram_tensor("bqc", [128, 4], f32, kind="ExternalInput").ap(),
        "out": nc.dram_tensor("out", [L, J], bf16, kind="ExternalOutput").ap(),
    }
    with tile.TileContext(nc) as tc:
        _emit(tc, aps)
    nc.compile()
    _CACHE["nc"] = nc
    return nc


def _pe():
    embed = np.arange(L, dtype=np.float32)
    dim_t = np.arange(D, dtype=np.float32)
    dim_t = (np.float32(TEMPERATURE) ** (2.0 * np.floor(dim_t / 2.0) / np.float32(D))).astype(np.float32)
    pos = embed[:, None] / dim_t  # [L, D]
    return np.stack([np.sin(pos[:, 0::2]), np.cos(pos[:, 1::2])], axis=2).reshape(L, D)


def _pmaj(a2d, inner):
    """[R, C] (R = t*128) -> partition-major [128, R//128 * C]."""
    t = a2d.shape[0] // 128
    return np.ascontiguousarray(
        a2d.reshape(t, 128, inner).transpose(1, 0, 2).reshape(128, t * inner)
    )


def _pmaj_lh(a2d):
    """[256, 1024] -> [128, (lhalf, t, 512)] partition-major, l-half-major."""
    a = a2d.reshape(2, 128, 2, 512)          # [t, p, lh, 512]
    return np.ascontiguousarray(a.transpose(1, 2, 0, 3).reshape(128, 2048))


def kernel(**inputs):
    global LAST_RESULT
    bf = np.float16
    x = np.asarray(inputs["x"], dtype=np.float32)
    wq = _pmaj(np.asarray(inputs["Wq"], dtype=np.float32).astype(bf), J)
    wk = _pmaj(np.asarray(inputs["Wk"], dtype=np.float32).astype(bf), J)
    wv = _pmaj(np.asarray(inputs["Wv"], dtype=np.float32).astype(bf), J)
    bq = np.asarray(inputs["bq"], dtype=np.float32)
    bv = np.asarray(inputs["bv"], dtype=np.float32)

    nc = _build()
    bqc = np.ascontiguousarray(np.repeat(bq, HD).reshape(4, 128).T)  # [128, 4]
    pe = _pe()  # [L, D] fp32
    qkp_all = (x + pe[None]).transpose(0, 2, 1).astype(bf)   # [B, D, L]
    xt_all = x.transpose(0, 2, 1).astype(bf)                 # [B, D, L]
    base = {"wq": wq, "wk": wk, "wv": wv, "bqc": bqc}
    in_maps = [
        {
            **base,
            "qkp": _pmaj_lh(qkp_all[b]),
            "xt": _pmaj(xt_all[b], L),
        }
        for b in range(B)
    ]
    res = run_bass_kernel_spmd(
        nc, in_maps, core_ids=list(range(B)), trace=TRACE
    )
    LAST_RESULT = res
    out = np.stack([res.results[b]["out"] for b in range(B)]).astype(np.float32)
    out += np.repeat(bv, HD)[None, None, :]
    return out


# revision 35
# speedup vs baseline: 1.0024x; 1.0024x over previous
"""Multi-head distance (attention) layer on 8 TRN2 NeuronCores.

Sharding: data-parallel over batch. B=8 -> one batch element per core.
Each core computes a full multi-head self-attention for its [L=1024, D=256]
slice with H=8 heads of dim 64. No collectives needed.

The kernel is ScalarE(ACT)-throughput-bound: softmax needs exp on all
H*L*L = 8.4M scores and ACT is the only engine with exp, at 128 lanes x
1.2 GHz => ~55us floor.  Everything is organized around keeping ACT 100%
busy doing nothing but exp:
  - The host ships x already TRANSPOSED, both with the positional
    encoding added in fp32 (qkpT = (x+pe).T, feeds Q/K) and without
    (xT, feeds V).  That kills the on-device transposes and pos-enc
    adds and makes every input DMA a single large contiguous transfer
    (one InstDMACopy stripes across all 16 SDMA engines).
  - PSUM: 6 banks are an S-score rotation (2 tiles x [128, 1536] fp32,
    i.e. 3 matmul chunks of 512 per exp) so each ACTIVATE amortizes its
    352-cycle fixed overhead over 1536 elements; the other 2 banks are a
    shared pool for QKV-projection / O-quad matmul outputs.
  - ACT executes ONLY the exp stream (plus input DMA triggers and one
    table preload, all finished before the first exp); every PSUM
    drain/copy lives on DVE (gpsimd/Pool cannot touch PSUM).
  - S matmuls use 64x128 PE row-tiling (tile_position): head 2j streams
    through PE rows 0-63 while head 2j+1 streams rows 64-127
    concurrently, so the d=64 contraction runs at full rate with no
    zero-padded K and no zeroed kT copies.
  - The S/exp stream is software-pipelined with the O quads of the
    previous head pair at half-S-pair granularity so the in-order PE
    queue never runs ACT dry; only the last two O quads (which consume
    the final exps) trail the exp stream.
Per-core algorithm (all matmul operands fp16: 1 col/cycle on the PE with
~fp32-grade mantissa for this problem's value ranges):
  qT   = Wq.T @ qkpT + bq       via matmul(lhsT=Wq, rhs=qkpT), DVE drain
  kTp  = Wk.T @ qkpT            per head-pair [128=2x64 d, 1024 m]
  v    = xT.T @ Wv              via matmul(lhsT=xT, rhs=Wv)
  per head pair (2j, 2j+1), interleaved chunk stream:
    sT[m,l] = sum_d kTp[d,m] qT[d,l]     row-tiled matmul chunks
    eT      = exp(0.125 * sT)            ACT, PSUM->SBUF, fp16, [128,1536]
    O[l,d]+Z = eT.T @ [v_h | 1]          matmul(lhsT=eT, rhs=v_aug), 4 output
                                         column-groups share one PSUM bank
    out_h   = O * (1/Z)                  DVE reciprocal + broadcast multiply,
                                         fp16 out_sb, DMA'd out per quad
Bias handling: bq added to qT during PSUM drain (per-partition scalar);
bk only shifts each score row by a constant (softmax-invariant) so it is
dropped; bv shifts the output by exactly repeat(bv, 64) because softmax
rows sum to 1, added on the host (which also upcasts the fp16 result).
"""

import numpy as np

import concourse.bass as bass
import concourse.mybir as mybir
import concourse.tile as tile
from concourse import bacc
from concourse.bass_utils import run_bass_kernel_spmd

B, L, D = 8, 1024, 256
H, HD = 8, 64
J = H * HD  # 512
TEMPERATURE = 10000.0

f32 = mybir.dt.float32
bf16 = mybir.dt.float16  # fp16: same PE rate as bf16, 8x the mantissa

_CACHE = {}
LAST_RESULT = None  # BassKernelResults of the most recent run (for profiling)
TRACE = False

STILE = 1536  # S-chunk PSUM/exp tile width (3 chunks of 512)


def _emit(tc, aps):
    nc = tc.nc
    Exp = mybir.ActivationFunctionType.Exp
    qkp, xt, wq, wk, wv, bqc, out = (
        aps["qkp"], aps["xt"], aps["wq"], aps["wk"], aps["wv"], aps["bqc"],
        aps["out"],
    )

    # all inputs arrive partition-major from the host: one contiguous
    # 2-4KB run per partition -> one InstDMACopy each with max-size
    # descriptors striped over the 16 SDMA engines.  qkp additionally
    # arrives l-half-major ([p][(lhalf, t, 512)]) so the first projection
    # pieces unblock after half the transfer.
    qkpr = qkp.rearrange("p (h t l) -> p h t l", t=2, l=512)  # [128,2,2,512]
    xtr = xt.rearrange("p (t l) -> p t l", l=1024)       # [128, 2, 1024]
    wqr = wq.rearrange("p (t j) -> p t j", j=512)        # [128, 2, 512]
    wkr = wk.rearrange("p (t j) -> p t j", j=512)
    wvr = wv.rearrange("p (t j) -> p t j", j=512)
    outr = out.rearrange("(n p) j -> p n j", p=128)      # [128, 8, 512]

    import contextlib
    ctx = contextlib.ExitStack()
    persist = ctx.enter_context(tc.tile_pool(name="persist", bufs=1))
    epool = ctx.enter_context(tc.tile_pool(name="epool", bufs=16))
    rpool = ctx.enter_context(tc.tile_pool(name="rpool", bufs=4))
    s_ps = ctx.enter_context(tc.tile_pool(name="sps", bufs=2, space="PSUM"))
    o_ps = ctx.enter_context(tc.tile_pool(name="ops", bufs=2, space="PSUM"))

    # --- input DMAs: one large contiguous transfer each, spread over the
    # three DMA-capable queues, critical-path operands (qkp, wk, wq) first.
    # qkp is split by L-half so the first k/q projection pieces (which only
    # read l 0:512) fire one transfer earlier.
    qkT = persist.tile([128, 2, 1024], bf16, name="qkT")
    xT = persist.tile([128, 2, 1024], bf16, name="xT")
    w_sb = {
        wname: persist.tile([128, 2, 512], bf16, name=f"{wname}_sb")
        for wname in ("wq", "wk", "wv")
    }
    bq_sb = persist.tile([128, 4], f32, name="bq_sb")

    # qkT SBUF layout is [128, t, l]; the lo DMA fills [:, :, 0:512] from
    # the contiguous lo block, hi fills [:, :, 512:1024]
    # wk/wq ride the two fast HWDGE rings right behind qkp-lo (SWDGE/Pool
    # transfers land ~2.5us later and would gate the first projections);
    # gpsimd only carries the tiny bq plus late-needed xt.
    nc.sync.dma_start(out=qkT[:, :, 0:512], in_=qkpr[:, 0])
    nc.scalar.dma_start(out=w_sb["wk"][:], in_=wkr[:])
    nc.sync.dma_start(out=w_sb["wq"][:], in_=wqr[:])
    nc.gpsimd.dma_start(out=bq_sb[:], in_=bqc[:, :])
    nc.sync.dma_start(out=qkT[:, :, 512:1024], in_=qkpr[:, 1])
    nc.scalar.dma_start(out=w_sb["wv"][:], in_=wvr[:])
    nc.gpsimd.dma_start(out=xT[:], in_=xtr[:])

    # --- ACT exp-table preload (after ACT's DMA triggers, before first exp)
    sc_in = persist.tile([128, 8], f32, name="sc_in")
    sc_out = persist.tile([128, 8], f32, name="sc_out")
    nc.vector.memset(sc_in[:], 0.0)
    nc.scalar.activation(sc_out[:], sc_in[:], Exp)

    # --- QKV projections (o-pool PSUM, drains on DVE) ---
    kTp = [persist.tile([128, 1024], bf16, name=f"kTp{j}") for j in range(4)]
    qT = [persist.tile([128, 1024], bf16, name=f"qT{j}") for j in range(4)]
    v_sb = [persist.tile([128, 8, 65], bf16, name=f"v_sb{m}") for m in range(8)]

    def qk_piece(j, which, l2):
        wname = "wq" if which == "q" else "wk"
        pq = o_ps.tile([128, 512], f32, tag="o", name="pq")
        for c2 in range(2):
            nc.tensor.matmul(
                pq[:, 0:512],
                lhsT=w_sb[wname][:, c2, j * 128:(j + 1) * 128],
                rhs=qkT[:, c2, l2 * 512:(l2 + 1) * 512],
                start=(c2 == 0),
                stop=(c2 == 1),
            )
        dsl = slice(l2 * 512, (l2 + 1) * 512)
        if which == "q":
            nc.vector.tensor_scalar_add(
                qT[j][:, dsl], pq[:, 0:512], bq_sb[:, j:j + 1]
            )
        else:
            nc.vector.tensor_copy(kTp[j][:, dsl], pq[:, 0:512])

    def v_proj(m):
        pv = o_ps.tile([128, 512], f32, tag="o", name="pv")
        for c2 in range(2):
            nc.tensor.matmul(
                pv[:, 0:512],
                lhsT=xT[:, c2, m * 128:(m + 1) * 128],
                rhs=w_sb["wv"][:, c2, :],
                start=(c2 == 0),
                stop=(c2 == 1),
            )
        nc.vector.tensor_copy(
            v_sb[m][:, :, 0:64], pv[:, 0:512].rearrange("p (h d) -> p h d", h=8)
        )
        nc.vector.memset(v_sb[m][:, :, 64:65], 1.0)

    # --- attention: S-chunks packed into [128, STILE] PSUM tiles; one exp
    # per tile. ---
    out_sb = persist.tile([128, 8, 512], bf16, name="out_sb")
    epos = {}  # (h, mc, l2) -> (e_tile, col_offset)
    state = {"tile": None, "off": 0, "chunks": []}

    def flush_exp():
        if state["tile"] is None or not state["chunks"]:
            return
        e = epool.tile([128, state["off"]], bf16, tag="e", name="e")
        nc.scalar.activation(
            e[:], state["tile"][:, 0:state["off"]], Exp, scale=float(HD) ** -0.5
        )
        for key, off in state["chunks"]:
            epos[key] = (e, off)
        state["tile"] = None
        state["off"] = 0
        state["chunks"] = []

    def s_chunk(h, mc, l2):
        if state["tile"] is None:
            state["tile"] = s_ps.tile([128, STILE], f32, tag="s", name="ps")
        off = state["off"]
        j, half = h >> 1, h & 1
        hp = slice(half * 64, half * 64 + 64)
        nc.tensor.matmul(
            state["tile"][:, off:off + 512],
            lhsT=kTp[j][hp, mc * 128:(mc + 1) * 128],
            rhs=qT[j][hp, l2 * 512:(l2 + 1) * 512],
            start=True,
            stop=True,
            tile_position=(half * 64, 0),
        )
        state["chunks"].append(((h, mc, l2), off))
        state["off"] = off + 512
        if state["off"] == STILE:
            flush_exp()

    def emit_S_sub(p, l2, mcs, heads=None):
        # heads=None: pair-interleaved (row-tiled concurrency); otherwise a
        # single head's chunks (used for the final pair so O(6,1) is gated
        # on an exp three tiles before the last one)
        for mc in mcs:
            for h in ([2 * p, 2 * p + 1] if heads is None else heads):
                s_chunk(h, mc, l2)

    pO_open = {}

    def emit_O_mms(h, q, gs, pO):
        for g in gs:
            lc = 4 * q + g
            l2, sub = lc // 4, lc % 4
            for mc in range(8):
                e, off = epos[(h, mc, l2)]
                nc.tensor.matmul(
                    pO[:, 65 * g:65 * g + 65],
                    lhsT=e[:, off + sub * 128:off + (sub + 1) * 128],
                    rhs=v_sb[mc][:, h, :],
                    start=(mc == 0),
                    stop=(mc == 7),
                )

    def emit_O_half(h, q, part):
        if part == 0:
            pO = o_ps.tile([128, 260], f32, tag="o", name="pO")
            pO_open[(h, q)] = pO
            emit_O_mms(h, q, (0, 1), pO)
        else:
            pO = pO_open.pop((h, q))
            emit_O_mms(h, q, (2, 3), pO)
            emit_O_fin(h, q, pO)

    def emit_O_fin(h, q, pO):
        hsl = slice(h * 64, (h + 1) * 64)
        pOr = pO.rearrange("p (g c) -> p g c", g=4)      # [128, 4, 65]
        rc = rpool.tile([128, 4], f32, tag="rc", name="rc")
        nc.vector.reciprocal(rc[:], pOr[:, :, 64])
        rcb = bass.AP(
            tensor=rc.tensor, offset=rc.offset,
            ap=[rc.ap[0], rc.ap[1], [0, 64]],
        )
        nc.vector.tensor_mul(
            out_sb[:, 4 * q:4 * q + 4, hsl], pOr[:, :, 0:64], rcb
        )
        if h == 7 and q == 1:
            for g2, eng in ((slice(4, 6), nc.sync), (slice(6, 8), nc.scalar)):
                eng.dma_start(out=outr[:, g2, hsl], in_=out_sb[:, g2, hsl])
        elif h == 7:
            for g2, eng in ((slice(0, 2), nc.sync), (slice(2, 4), nc.gpsimd)):
                eng.dma_start(out=outr[:, g2, hsl], in_=out_sb[:, g2, hsl])
        elif h == 6 and q == 1:
            for g2, eng in ((slice(4, 6), nc.sync), (slice(6, 8), nc.scalar)):
                eng.dma_start(out=outr[:, g2, hsl], in_=out_sb[:, g2, hsl])
        else:
            eng = nc.sync if q == 0 else nc.gpsimd
            eng.dma_start(
                out=outr[:, 4 * q:4 * q + 4, hsl],
                in_=out_sb[:, 4 * q:4 * q + 4, hsl],
            )

    def emit_O_quad(h, q):
        pO = o_ps.tile([128, 260], f32, tag="o", name="pO")
        emit_O_mms(h, q, (0, 1, 2, 3), pO)
        emit_O_fin(h, q, pO)

    # schedule: S emission in 4-chunk sub-steps with the previous pair's O
    # half-quads and the next pair's QKV projections dropped in between, so
    # the in-order PE queue always has S fills near the head and ACT never
    # runs dry.  The final pair's l2=1 chunks go head-6-first so O(6,1)
    # unblocks three exps before the end; only O(7,1) trails the final exp.
    qk_piece(0, "k", 0)
    qk_piece(0, "q", 0)
    emit_S_sub(0, 0, (0, 1))
    qk_piece(0, "k", 1)
    emit_S_sub(0, 0, (2, 3))
    qk_piece(0, "q", 1)
    emit_S_sub(0, 0, (4, 5))
    qk_piece(1, "k", 0)
    emit_S_sub(0, 0, (6, 7))
    qk_piece(1, "k", 1)
    emit_S_sub(0, 1, (0, 1))
    v_proj(0)
    emit_S_sub(0, 1, (2, 3))
    v_proj(1)
    qk_piece(1, "q", 0)
    emit_S_sub(0, 1, (4, 5))
    v_proj(2)
    qk_piece(1, "q", 1)
    emit_S_sub(0, 1, (6, 7))
    for m in range(3, 8):
        v_proj(m)
    for p in range(1, 4):
        quads = [(2 * p - 2, 0), (2 * p - 2, 1), (2 * p - 1, 0), (2 * p - 1, 1)]
        inj = [("k", 0), ("k", 1), ("q", 0), ("q", 1)] if p < 3 else None
        subs = (
            [(0, (0, 1), None), (0, (2, 3), None), (0, (4, 5), None),
             (0, (6, 7), None), (1, (0, 1), None), (1, (2, 3), None),
             (1, (4, 5), None), (1, (6, 7), None)]
            if p < 3 else
            [(0, (0, 1), None), (0, (2, 3), None), (0, (4, 5), None),
             (0, (6, 7), None), (1, (0, 1, 2, 3), [6]), (1, (4, 5, 6, 7), [6]),
             (1, (0, 1, 2, 3), [7]), (1, (4, 5, 6, 7), [7])]
        )
        for subi, (l2, mcs, heads) in enumerate(subs):
            emit_S_sub(p, l2, mcs, heads)
            h, q = quads[subi // 2]
            emit_O_half(h, q, subi % 2)
            if inj is not None and subi % 2 == 1:
                w, l2i = inj[subi // 2]
                qk_piece(p + 1, w, l2i)
    # O(6,0)/O(7,0) consume l2=0 exps (long done) and run during the final
    # exps; O(6,1) is gated three exps early, O(7,1) on the final exp.
    flush_exp()
    emit_O_quad(6, 0)
    emit_O_quad(7, 0)
    emit_O_quad(6, 1)
    emit_O_quad(7, 1)
    ctx.close()


def _build():
    if "nc" in _CACHE:
        return _CACHE["nc"]
    nc = bacc.Bacc("TRN2", target_bir_lowering=False, debug=False, num_devices=8)
    aps = {
        "qkp": nc.dram_tensor("qkp", [128, 2 * L], bf16, kind="ExternalInput").ap(),
        "xt": nc.dram_tensor("xt", [128, 2 * L], bf16, kind="ExternalInput").ap(),
        "wq": nc.dram_tensor("wq", [128, 2 * J], bf16, kind="ExternalInput").ap(),
        "wk": nc.dram_tensor("wk", [128, 2 * J], bf16, kind="ExternalInput").ap(),
        "wv": nc.dram_tensor("wv", [128, 2 * J], bf16, kind="ExternalInput").ap(),
        "bqc": nc.dram_tensor("bqc", [128, 4], f32, kind="ExternalInput").ap(),
        "out": nc.dram_tensor("out", [L, J], bf16, kind="ExternalOutput").ap(),
    }
    with tile.TileContext(nc) as tc:
        _emit(tc, aps)
    nc.compile()
    _CACHE["nc"] = nc
    return nc


def _pe():
    embed = np.arange(L, dtype=np.float32)
    dim_t = np.arange(D, dtype=np.float32)
    dim_t = (np.float32(TEMPERATURE) ** (2.0 * np.floor(dim_t / 2.0) / np.float32(D))).astype(np.float32)
    pos = embed[:, None] / dim_t  # [L, D]
    return np.stack([np.sin(pos[:, 0::2]), np.cos(pos[:, 1::2])], axis=2).reshape(L, D)


def _pmaj(a2d, inner):
    """[R, C] (R = t*128) -> partition-major [128, R//128 * C]."""
    t = a2d.shape[0] // 128
    return np.ascontiguousarray(
        a2d.reshape(t, 128, inner).transpose(1, 0, 2).reshape(128, t * inner)
    )


def _pmaj_lh(a2d):
    """[256, 1024] -> [128, (lhalf, t, 512)] partition-major, l-half-major."""
    a = a2d.reshape(2, 128, 2, 512)          # [t, p, lh, 512]
    return np.ascontiguousarray(a.transpose(1, 2, 0, 3).reshape(128, 2048))


def kernel(**inputs):
    global LAST_RESULT
    bf = np.float16
    x = np.asarray(inputs["x"], dtype=np.float32)
    wq = _pmaj(np.asarray(inputs["Wq"], dtype=np.float32).astype(bf), J)
    wk = _pmaj(np.asarray(inputs["Wk"], dtype=np.float32).astype(bf), J)
    wv = _pmaj(np.asarray(inputs["Wv"], dtype=np.float32).astype(bf), J)
    bq = np.asarray(inputs["bq"], dtype=np.float32)
    bv = np.asarray(inputs["bv"], dtype=np.float32)

    nc = _build()
    bqc = np.ascontiguousarray(np.repeat(bq, HD).reshape(4, 128).T)  # [128, 4]
    pe = _pe()  # [L, D] fp32
    qkp_all = (x + pe[None]).transpose(0, 2, 1).astype(bf)   # [B, D, L]
    xt_all = x.transpose(0, 2, 1).astype(bf)                 # [B, D, L]
    base = {"wq": wq, "wk": wk, "wv": wv, "bqc": bqc}
    in_maps = [
        {
            **base,
            "qkp": _pmaj_lh(qkp_all[b]),
            "xt": _pmaj(xt_all[b], L),
        }
        for b in range(B)
    ]
    res = run_bass_kernel_spmd(
        nc, in_maps, core_ids=list(range(B)), trace=TRACE
    )
    LAST_RESULT = res
    out = np.stack([res.results[b]["out"] for b in range(B)]).astype(np.float32)
    out += np.repeat(bv, HD)[None, None, :]
    return out
